# revision 24
# baseline (speedup 1.0000x reference)
"""Fused CE + supervised-contrastive loss on 8 Trainium2 NeuronCores.

Math (reference semantics):
  ce   = -mean_i log_softmax(input)[i, y_i]
  sim  = (X @ X.T) / tau, diag excluded
  lse_i = logsumexp_{k!=i} sim[i,k]
  possum_i = (x_i . S_{y_i} - ||x_i||^2)/tau, S_c = sum_{k: y_k=c} x_k
  per_i = lse_i - possum_i/n_pos_i  (0 if n_pos_i == 0)
  loss = (1-lmbd)*ce + lmbd * sum_i per_i

The ONLY O(N^2) term is the row-wise sum of exp(sim); the device computes
exactly that (8.4M exps/core), split across two engines working in
parallel on a shared 3-slot PSUM rotation:

  * ACT path (local cols 0..ACOLS): row-major sim chunks [128 rows, 1024].
    ScalarE exp with bias=-SHIFT and fused accum_out row-sums (esum).
    The diagonal (always in local cols [0,1024) thanks to the per-core xt2
    rotation) is killed pre-exp by a diag(-1e4) accumulate-matmul.
  * DVE path (cols ACOLS..8192): TRANSPOSED sim chunks [128 cols, 1024
    rows] (lhsT = xt2 column block, rhs = xbt). VectorE computes a
    one-instruction Schraudolph fast-exp: bits16 = int16(max(sim,0) *
    128/ln2), bitcast to bf16 == e^(sim - 127*ln2) * rho, rho in
    [1, 1.086] (measured mean 1.0410 on HW, folded into KDV). The clamp is
    mandatory: the int16 convert WRAPS on negative; clamped terms
    contribute exactly +0.0 (true value < e^-88, negligible).
    The PE row-sums the bf16 tiles with masked-ones lhsT matmuls (mask
    columns 0-63 select out-rows 0-63 for row-half 0, 64-127 for half 1)
    so BOTH halves accumulate into one standard [128, 512] PSUM bank at
    tile_position (0,0) -- col-tiled M=1 outputs proved broken. A 4x
    PE-transpose at the tail converts rows-in-free to row-major [128, 8].

  PE lanes: ACT fills use lo operand copies (array rows 0-63), DVE fills
  hi copies (rows 64-127), emitted interleaved for row-group overlap.
  Each chunk's reduce matmuls are deferred one super-step so the in-order
  PE queue never waits on DVE's fast-exp.

Everything O(N*C) -- class sums, positive-pair sums, counts, row norms,
logit gather, the CE term, and the final combine -- runs on the host in
float64 alongside the input prep (rotations/onehots/casts). This removes
the AllReduce whose enqueue-barrier (~47us of cross-core skew) + ~14us
ncfw latency gated the previous design's tail.

Outputs per core: [128, 8] se_act (shift-100 domain) and [128, 8] seD
(fast-exp domain); host computes lse = ln(se + KDV*seD) + SHIFT.
"""

import math

import numpy as np

N, C = 8192, 64
NCORES = 8
RPC = N // NCORES          # rows per core (1024)
P = 128                    # partitions per row-block
NBLK = RPC // P            # 8 row blocks per core
TAU = 0.5
LMBD = 0.5
SHIFT = 100.0              # ACT-path logsumexp shift
ACOLS = 4096               # ACT row-major columns per core
DCOLS = N - ACOLS          # DVE transposed columns per core
AW = 1024                  # chunk width
NACH = ACOLS // AW         # ACT chunks per block (4)
NKB = DCOLS // P           # DVE column blocks (32)
AEXP = 128.0 / math.log(2.0)          # fast-exp scale (184.6646)
RHO = 1.0410                          # measured mean Schraudolph ratio (HW)
KDV = math.exp(127.0 * math.log(2.0) - SHIFT) / RHO

_CACHE = {}


def _build():
    from contextlib import ExitStack

    import concourse.bass as bass
    import concourse.tile as tile
    from concourse import bacc, mybir

    f32 = mybir.dt.float32
    i16 = mybir.dt.int16
    bf16 = mybir.dt.bfloat16
    AF = mybir.ActivationFunctionType
    ALU = mybir.AluOpType
    AX = mybir.AxisListType

    nc = bacc.Bacc(
        "TRN2",
        target_bir_lowering=False,
        debug=False,
        num_devices=NCORES,
    )

    xt2a_d = nc.dram_tensor("xt2a", [C, ACOLS], bf16, kind="ExternalInput")
    xt2b_d = nc.dram_tensor("xt2b", [C, DCOLS], bf16, kind="ExternalInput")
    xbt_d = nc.dram_tensor("xbt", [C, RPC], bf16, kind="ExternalInput")
    eye_d = nc.dram_tensor("eyeneg", [P, P], bf16, kind="ExternalInput")
    idn_d = nc.dram_tensor("ident", [P, P], bf16, kind="ExternalInput")
    out_d = nc.dram_tensor("out", [P, 16], f32, kind="ExternalOutput")

    def emit(tc, ctx):
        const = ctx.enter_context(tc.tile_pool(name="const", bufs=1))
        strm = ctx.enter_context(tc.tile_pool(name="strm", bufs=3, space="PSUM"))
        accp = ctx.enter_context(tc.tile_pool(name="accp", bufs=1, space="PSUM"))
        auxp = ctx.enter_context(tc.tile_pool(name="auxp", bufs=1, space="PSUM"))
        scrp = ctx.enter_context(tc.tile_pool(name="scrp", bufs=2))
        ep = ctx.enter_context(tc.tile_pool(name="ep", bufs=3))
        stats = ctx.enter_context(tc.tile_pool(name="stats", bufs=1))

        # ---- input DMAs; high_priority hints the scheduler to place
        # them ahead of the semaphore-reset preamble on the sync queue ----
        hp = tc.high_priority()
        hp.__enter__()
        xbt_sb = const.tile([P, RPC], bf16)
        nc.sync.dma_start(xbt_sb[0:C, :], xbt_d.ap())
        nc.sync.dma_start(xbt_sb[C:P, :], xbt_d.ap())
        xt2b_sb = const.tile([P, DCOLS], bf16)
        nc.sync.dma_start(xt2b_sb[C:P, 0:AW], xt2b_d.ap()[:, 0:AW])
        xt2a_sb = const.tile([P, ACOLS], bf16)
        nc.sync.dma_start(xt2a_sb[0:C, 0:AW], xt2a_d.ap()[:, 0:AW])
        eye_sb = const.tile([P, P], bf16)
        nc.sync.dma_start(eye_sb[:], eye_d.ap())
        idn_sb = const.tile([P, P], bf16)
        nc.sync.dma_start(idn_sb[:], idn_d.ap())
        nc.sync.dma_start(xt2a_sb[0:C, AW:ACOLS], xt2a_d.ap()[:, AW:ACOLS])
        nc.sync.dma_start(xt2b_sb[C:P, AW:DCOLS], xt2b_d.ap()[:, AW:DCOLS])
        hp.__exit__(None, None, None)

        # ---- persistent small tiles ----
        nshift = stats.tile([P, 1], f32)
        nc.vector.memset(nshift[:], -SHIFT)
        # masked-ones lhsT: mh0 -> out rows 0-63 (row-half 0), mh1 -> 64-127
        masks = stats.tile([P, 2 * P], bf16)
        nc.vector.memset(masks[:, 0:C], 1.0)
        nc.vector.memset(masks[:, C:P], 0.0)
        nc.vector.memset(masks[:, P : P + C], 0.0)
        nc.vector.memset(masks[:, P + C : 2 * P], 1.0)
        acc_sb = stats.tile([P, 512], bf16)
        res = stats.tile([P, 16], f32)

        # rowsum accumulator: rows 0-63 = row-half 0 (redundant copies),
        # rows 64-127 = half 1; free = row-within-half
        acc = accp.tile([P, 512], f32, tag="acc")
        # ACT accumulator drains to PSUM (faster ScE port than SBUF); the
        # aux bank is otherwise idle until the tail
        esum = auxp.tile([P, 512], f32, tag="aux")

        # ---- main interleaved exp stream ----
        pending_acc = []

        def emit_acc(kb, eb):
            for h in range(2):
                nc.tensor.matmul(
                    acc[:, 0:512],
                    lhsT=masks[:, h * P : (h + 1) * P],
                    rhs=eb[:, h * 512 : (h + 1) * 512],
                    start=(kb == 0 and h == 0),
                    stop=(kb == NKB - 1 and h == 1),
                    skip_group_check=True,
                )

        def super_step(b, w, kb):
            ps_a = strm.tile([P, AW], f32, tag="s")
            ps_d = strm.tile([P, AW], f32, tag="s")
            for h in range(2):
                nc.tensor.matmul(
                    ps_d[:, h * 512 : (h + 1) * 512],
                    lhsT=xt2b_sb[C:P, kb * P : (kb + 1) * P],
                    rhs=xbt_sb[C:P, h * 512 : (h + 1) * 512],
                    start=True,
                    stop=True,
                )
                nc.tensor.matmul(
                    ps_a[:, h * 512 : (h + 1) * 512],
                    lhsT=xbt_sb[0:C, b * P : (b + 1) * P],
                    rhs=xt2a_sb[0:C, w * AW + h * 512 : w * AW + (h + 1) * 512],
                    start=True,
                    stop=True,
                )
            if w == 0:
                # kill self-similarity (local col b*128+p) pre-exp
                nc.tensor.matmul(
                    ps_a[:, b * P : (b + 1) * P],
                    lhsT=idn_sb[:],
                    rhs=eye_sb[:],
                    start=False,
                    stop=True,
                    skip_group_check=True,
                )
            if pending_acc:
                emit_acc(*pending_acc.pop())
            scr = scrp.tile([P, AW], bf16, tag="scr")
            idx = b * NACH + w
            nc.scalar.activation(
                scr[:], ps_a[:], AF.Exp, bias=nshift[:],
                accum_out=esum[:, idx : idx + 1],
            )
            et = ep.tile([P, AW], i16, tag="E")
            nc.vector.tensor_scalar(
                out=et[:], in0=ps_d[:], scalar1=0.0, scalar2=AEXP,
                op0=ALU.max, op1=ALU.mult,
            )
            pending_acc.append((kb, et[:].bitcast(bf16)))

        a_list = [(b, w) for b in range(NBLK) for w in range(NACH)]
        for step in range(NKB):
            b, w = a_list[step]
            super_step(b, w, step)
        while pending_acc:
            emit_acc(*pending_acc.pop())

        # ---- tail: per-block rowsums out ----
        nc.vector.reduce_sum(
            res[:, 0:NBLK],
            esum[:, 0 : NBLK * NACH].rearrange("p (b w) -> p b w", w=NACH),
            axis=AX.X,
        )
        nc.vector.tensor_copy(acc_sb[:], acc[:])
        tps = accp.tile([P, 512], bf16, tag="acc")
        for w in range(4):
            nc.tensor.transpose(
                tps[:, w * P : (w + 1) * P],
                acc_sb[:, w * P : (w + 1) * P],
                idn_sb[:],
            )
        # seD[p, b] with b = h*4 + w sits at tps[p, w*128 + h*64]
        tq = tps[:].rearrange("p (w q r) -> p w q r", w=4, q=2)
        seD_v = res[:, NBLK : 2 * NBLK].rearrange("p (h w o) -> p w h o", w=4, o=1)
        nc.vector.tensor_copy(seD_v, tq[:, :, 0:2, 0:1])
        nc.sync.dma_start(out_d.ap(), res[:])

    with tile.TileContext(nc) as tc, ExitStack() as ctx:
        emit(tc, ctx)

    nc.compile()
    return nc


def _get_nc(**kw):
    key = repr(sorted(kw.items()))
    if key not in _CACHE:
        _CACHE[key] = _build(**kw)
    return _CACHE[key]


def _make_in_maps(X, y):
    import ml_dtypes

    bf = ml_dtypes.bfloat16
    X = np.ascontiguousarray(np.asarray(X, dtype=np.float32))
    assert X.shape == (N, C)

    xt2 = np.ascontiguousarray((X.T / np.float32(TAU)).astype(bf))
    eyeneg = (np.eye(P) * -1e4).astype(bf)
    ident = np.eye(P).astype(bf)

    in_maps = []
    for r in range(NCORES):
        rows = slice(r * RPC, (r + 1) * RPC)
        xb = X[rows]
        xt2r = np.roll(xt2, -r * RPC, axis=1)
        in_maps.append(
            {
                "xt2a": np.ascontiguousarray(xt2r[:, :ACOLS]),
                "xt2b": np.ascontiguousarray(xt2r[:, ACOLS:]),
                "xbt": np.ascontiguousarray(xb.T.astype(bf)),
                "eyeneg": eyeneg,
                "ident": ident,
            }
        )
    return in_maps


def run(input, target, trace=False, **build_kw):
    """Run the device kernel; returns (loss_scalar, BassKernelResults)."""
    from concourse.bass_utils import run_bass_kernel_spmd

    nc = _get_nc(**build_kw)
    X = np.ascontiguousarray(np.asarray(input, dtype=np.float32))
    y = np.asarray(target).astype(np.int64).ravel()
    in_maps = _make_in_maps(X, y)
    res = run_bass_kernel_spmd(
        nc, in_maps, core_ids=list(range(NCORES)), trace=trace
    )

    # device gave per-row exp sums; the O(N*C) remainder runs here in f64
    se = np.empty(N)
    for r, core_out in enumerate(res.results):
        o = core_out["out"].astype(np.float64)  # [128, 16]
        se_act = o[:, 0:NBLK]    # [128, 8], row b*128+p -> [p, b]
        se_dve = o[:, NBLK:16]
        tot = se_act + KDV * se_dve              # shift-SHIFT domain
        se[r * RPC:(r + 1) * RPC] = np.maximum(tot.T.ravel(), 1e-300)

    Xd = X.astype(np.float64)
    lse = np.log(se) + SHIFT                     # [N]
    S = np.zeros((C, C))
    np.add.at(S, y, Xd)
    counts = np.bincount(y, minlength=C).astype(np.float64)
    n_pos = counts[y] - 1.0
    possum = ((Xd * S[y]).sum(axis=1) - (Xd * Xd).sum(axis=1)) / TAU
    per_i = np.where(n_pos > 0, lse - possum / np.maximum(n_pos, 1.0), 0.0)
    sc = per_i.sum()

    m = Xd.max(axis=1)
    ce_lse = np.log(np.exp(Xd - m[:, None]).sum(axis=1)) + m
    ce = (ce_lse - Xd[np.arange(N), y]).mean()

    loss = (1.0 - LMBD) * ce + LMBD * sc
    return np.array(loss, dtype=np.float32), res


def kernel(input, target):
    loss, _ = run(input, target, trace=False)
    return loss
